# revision 1
# baseline (speedup 1.0000x reference)
"""Causal self-attention on 8 TRN2 NeuronCores.

Reference computation (B=4, T=2048, C=1024, H=16, D=64, fp32):
    qkv = x @ W_attn + b_attn ; split q,k,v ; per-head causal softmax(q k^T / 8) @ v
    y = heads @ W_proj + b_proj

Sharding: core c handles batch b = c//2 and head-half hh = c%2 (8 heads).
QKV weights are column-split and W_proj row-split per core, so each core
computes an independent partial projection; the host sums the two partials
per batch and adds the bias terms (b_proj and the folded-out v-bias
contribution b_v @ W_proj, which is constant because softmax rows sum to 1).
No collectives, no redundant FLOPs.

Per-core kernel layout trick: everything stays "transposed" so no PE
transposes are needed anywhere:
  - qT, kT are feature-major [head*64, T]; v is token-major with a ones
    column per head ([T, 8 x (64 v | 1)]).
  - S^T tiles [k=128, q<=512] come from K=64 matmuls, two heads run
    concurrently in the PE array via tile_position row groups (0,0)/(64,0).
  - exp reads both heads' PSUM banks in a single ACT op; causal masking is
    a DVE multiply with a host-sent triangle mask on the diagonal blocks
    only; above-diagonal work is skipped at tile granularity (suffix
    trimming).
  - A@V accumulates yT += v_aug^T @ expST in PSUM with M=65: row 64 is the
    softmax denominator (ones column). A DVE reciprocal of that row is
    broadcast across partitions by GPSIMD partition_broadcast (the Pool
    engine is otherwise idle), and one DVE multiply normalizes while
    writing the bf16 yT chunk used directly as the projection matmul's
    stationary operand.
  - Emission order doubles as scheduler priority: QKV groups of the next
    t-quarter and projection groups of finished quarters are interleaved
    between attention head-pairs, so the PE always has ready work while
    the ACT engine grinds through exp.

Dtypes: matmul inputs in bf16 (halves DMA; fp32 PSUM accumulation keeps the
end-to-end error ~2.6e-3 against the 2e-2 gate).
"""

import numpy as np
import ml_dtypes

import concourse.bacc as bacc
import concourse.mybir as mybir
import concourse.tile as tile
from concourse import library_config
from concourse.bass_utils import run_bass_kernel_spmd

F32 = mybir.dt.float32
F32R = mybir.dt.float32r
BF16 = mybir.dt.bfloat16
AF = mybir.ActivationFunctionType
ALU = mybir.AluOpType

N_CORES = 8
B, T, C = 4, 2048, 1024
H, D = 16, 64
CH = 512            # features per core (8 heads * 64)
NFO = 4             # head-pair chunks of 128 features
NTQ = 4             # t quarters of 512
NTC = 16            # t chunks of 128
SCALE = 0.125       # 1/sqrt(64)

_cached = {}


def _build_nc():
    nc = bacc.Bacc("TRN2", debug=False, num_devices=N_CORES)

    d_xT = nc.dram_tensor("xT", [C, T], BF16, kind="ExternalInput")
    d_wq = nc.dram_tensor("wq", [C, CH], BF16, kind="ExternalInput")
    d_wk = nc.dram_tensor("wk", [C, CH], BF16, kind="ExternalInput")
    d_wv = nc.dram_tensor("wv", [C, CH], BF16, kind="ExternalInput")
    d_bq = nc.dram_tensor("bq", [128, NFO], F32, kind="ExternalInput")
    d_bk = nc.dram_tensor("bk", [128, NFO], F32, kind="ExternalInput")
    d_wp = nc.dram_tensor("wp", [CH, C], BF16, kind="ExternalInput")
    d_masks = nc.dram_tensor("masks", [128, 1536], BF16, kind="ExternalInput")
    d_out = nc.dram_tensor("out", [T, C], F32, kind="ExternalOutput")

    with tile.TileContext(nc) as tc, nc.allow_low_precision(
        reason="bf16/f32r staging; accumulation stays fp32 in PSUM"
    ), (
        tc.tile_pool(name="persist", bufs=1)
    ) as persist, (
        tc.tile_pool(name="pW", bufs=1)
    ) as pW, (
        tc.tile_pool(name="pX", bufs=1)
    ) as pX, (
        tc.tile_pool(name="pO", bufs=4)
    ) as pO, (
        tc.tile_pool(name="p2e", bufs=6)
    ) as p2e, (
        tc.tile_pool(name="p2r", bufs=3)
    ) as p2r, (
        tc.tile_pool(name="psA", bufs=2, space="PSUM")
    ) as psA, (
        tc.tile_pool(name="psS", bufs=2, space="PSUM")
    ) as psS, (
        tc.tile_pool(name="psY", bufs=1, space="PSUM")
    ) as psY:
        # persistent on-chip tensors
        qT = [persist.tile([128, T], BF16, tag=f"qT{fo}", name=f"qT{fo}") for fo in range(NFO)]
        kT = [persist.tile([128, T], BF16, tag=f"kT{fo}", name=f"kT{fo}") for fo in range(NFO)]
        v = [persist.tile([128, 8, 65], BF16, tag=f"v{i}", name=f"v{i}") for i in range(NTC)]
        yT = [persist.tile([128, T], BF16, tag=f"yT{fo}", name=f"yT{fo}") for fo in range(NFO)]
        bq_sb = persist.tile([128, NFO], F32, tag="bq")
        bk_sb = persist.tile([128, NFO], F32, tag="bk")
        masks_sb = persist.tile([128, 1536], BF16, tag="masks")
        wq_sb = pW.tile([128, 8, CH], BF16, tag="wq")
        wk_sb = pW.tile([128, 8, CH], BF16, tag="wk")
        wv_sb = pW.tile([128, 8, CH], BF16, tag="wv")
        wp_sb = pW.tile([128, 4, C], BF16, tag="wp")
        x_tiles = [pX.tile([128, 8, 512], BF16, tag=f"x{tq}", name=f"x{tq}") for tq in range(NTQ)]

        # input DMAs: first-needed tensors split for a fast first matmul;
        # per-DMA sync-sequencer dispatch is ~650ns, so keep the count low
        nc.gpsimd.load_library(library_config.attn)

        def _w_piece(dst, src, c0, c1):
            nc.sync.dma_start(
                dst[:, c0:c1, :],
                src.ap()[128 * c0 : 128 * c1, :].rearrange("(c p) f -> p c f", p=128),
            )

        nc.sync.dma_start(
            x_tiles[0][:, 0:2, :],
            d_xT.ap()[0:256, 0:512].rearrange("(c p) t -> p c t", p=128),
        )
        _w_piece(wq_sb, d_wq, 0, 2)
        nc.sync.dma_start(bq_sb[:], d_bq.ap())
        nc.sync.dma_start(bk_sb[:], d_bk.ap())
        nc.sync.dma_start(
            x_tiles[0][:, 2:8, :],
            d_xT.ap()[256:1024, 0:512].rearrange("(c p) t -> p c t", p=128),
        )
        _w_piece(wq_sb, d_wq, 2, 8)
        _w_piece(wk_sb, d_wk, 0, 4)
        _w_piece(wv_sb, d_wv, 0, 4)
        _w_piece(wk_sb, d_wk, 4, 8)
        _w_piece(wv_sb, d_wv, 4, 8)
        nc.sync.dma_start(masks_sb[:], d_masks.ap())
        for tq in range(1, NTQ):
            nc.sync.dma_start(
                x_tiles[tq][:],
                d_xT.ap()[:, 512 * tq : 512 * (tq + 1)].rearrange("(c p) t -> p c t", p=128),
            )
        nc.sync.dma_start(wp_sb[:], d_wp.ap().rearrange("(c p) f -> p c f", p=128))

        def emit_qk_group(bq_, w_sb, b_sb, dst, fo):
            ps = psA.tile([128, 512], F32, tag="psA", name="ps_qk")
            for ci in range(8):
                nc.tensor.matmul(
                    ps[:],
                    w_sb[:, ci, 128 * fo : 128 * (fo + 1)],
                    x_tiles[bq_][:, ci, :],
                    start=(ci == 0),
                    stop=(ci == 7),
                )
            nc.vector.tensor_scalar(
                dst[fo][:, 512 * bq_ : 512 * (bq_ + 1)],
                ps[:],
                b_sb[:, fo : fo + 1],
                None,
                op0=ALU.add,
            )

        def emit_v_group(bq_, ts_):
            tci = 4 * bq_ + ts_
            ps = psA.tile([128, 512], F32, tag="psA", name="ps_v")
            for ci in range(8):
                nc.tensor.matmul(
                    ps[:],
                    x_tiles[bq_][:, ci, 128 * ts_ : 128 * (ts_ + 1)],
                    wv_sb[:, ci, :],
                    start=(ci == 0),
                    stop=(ci == 7),
                )
            nc.vector.memset(v[tci][:, :, 64:65], 1.0)
            nc.vector.tensor_copy(
                v[tci][:, :, 0:64],
                ps[:].rearrange("p (h d) -> p h d", h=8),
            )

        def emit_qkv_group(bq_, g):
            if g < 4:
                emit_qk_group(bq_, wq_sb, bq_sb, qT, g)
            elif g < 8:
                emit_qk_group(bq_, wk_sb, bk_sb, kT, g - 4)
            else:
                emit_v_group(bq_, g - 8)

        def emit_proj_tc(tci):
            o_sb = pO.tile([128, C], F32, tag="o", name="o_sb")
            for co in range(2):
                ps = psA.tile([128, 512], F32, tag="psA", name="ps_o")
                for fo in range(NFO):
                    nc.tensor.matmul(
                        ps[:],
                        yT[fo][:, 128 * tci : 128 * (tci + 1)],
                        wp_sb[:, fo, 512 * co : 512 * (co + 1)],
                        start=(fo == 0),
                        stop=(fo == 3),
                    )
                if co == 0:
                    nc.scalar.copy(o_sb[:, 0:512], ps[:])
                else:
                    nc.vector.tensor_copy(o_sb[:, 512:1024], ps[:])
                nc.sync.dma_start(
                    d_out.ap()[128 * tci : 128 * (tci + 1), 512 * co : 512 * (co + 1)],
                    o_sb[:, 512 * co : 512 * (co + 1)],
                )

        def emit_attn(fo, b):
            hA, hB = 2 * fo, 2 * fo + 1
            q0 = 512 * b
            pyA = psY.tile([128, 512], F32, tag="pyA")
            pyB = psY.tile([128, 512], F32, tag="pyB")
            # diagonal chunks first (kc = 4b+i), then full rows: the i=0
            # chunk covers the full 512-wide psum region with start=True,
            # the last chunk in the order carries stop=True (also full
            # width: for b=0 the diagonal chunks run full-width with wider
            # masks; for b>0 the last chunk is a full row).
            order = [4 * b + i for i in range(4)] + list(range(4 * b))
            for idx, kc in enumerate(order):
                i = kc - 4 * b  # >=0 for diagonal chunks
                qoff = 0 if (i <= 0 or b == 0) else 128 * i
                pS = psS.tile([128, 1024], F32, tag="pS", name="pS")
                nc.tensor.matmul(
                    pS[:, qoff:512],
                    kT[fo][0:64, 128 * kc : 128 * (kc + 1)],
                    qT[fo][0:64, q0 + qoff : q0 + 512],
                )
                nc.tensor.matmul(
                    pS[:, 512 + qoff : 1024],
                    kT[fo][64:128, 128 * kc : 128 * (kc + 1)],
                    qT[fo][64:128, q0 + qoff : q0 + 512],
                )
                eST = p2e.tile([128, 1024], BF16, tag="eST", name="eST")
                if qoff == 0:
                    nc.scalar.activation(eST[:], pS[:], AF.Exp, scale=SCALE)
                else:
                    # one ACT op over both heads' valid suffixes, skipping the
                    # [512, 512+qoff) hole via a strided AP
                    pS2 = pS[:].rearrange("p (two n) -> p two n", two=2)
                    eST2 = eST[:].rearrange("p (two n) -> p two n", two=2)
                    nc.scalar.activation(
                        eST2[:, :, qoff:512], pS2[:, :, qoff:512], AF.Exp, scale=SCALE
                    )
                if i >= 0:
                    # causal mask multiply on the diagonal part.
                    # masks_sb cols (bf16): [0,128) tri (keep j>=p),
                    # [128,384) [tri|1], [384,640) [0|tri],
                    # [640,1024) [0,0|tri], [1024,1536) [0,0,0|tri]
                    if b == 0:
                        moff, mw, eoff = [
                            (0, 128, 0),
                            (384, 256, 0),
                            (640, 384, 0),
                            (1024, 512, 0),
                        ][i]
                    else:
                        moff, mw, eoff = [
                            (0, 128, 0),
                            (0, 128, 128),
                            (128, 256, 256),
                            (0, 128, 384),
                        ][i]
                    for off in (eoff, 512 + eoff):
                        nc.vector.tensor_tensor(
                            eST[:, off : off + mw],
                            eST[:, off : off + mw],
                            masks_sb[:, moff : moff + mw],
                            op=ALU.mult,
                        )
                first = idx == 0
                last = idx == len(order) - 1
                nc.tensor.matmul(
                    pyA[0:65, qoff:512],
                    v[kc][:, hA, :],
                    eST[:, qoff:512],
                    start=first,
                    stop=last,
                )
                nc.tensor.matmul(
                    pyB[0:65, qoff:512],
                    v[kc][:, hB, :],
                    eST[:, 512 + qoff : 1024],
                    start=first,
                    stop=last,
                )
            # normalize: reciprocal of the ones-column denominator row, PE
            # broadcast over partitions, one DVE multiply per head into yT
            recA = p2r.tile([1, 512], F32, tag="recA", name="recA")
            recB = p2r.tile([1, 512], F32, tag="recB", name="recB")
            nc.vector.reciprocal(recA[:], pyA[64:65, :])
            nc.vector.reciprocal(recB[:], pyB[64:65, :])
            # broadcast across partitions on the otherwise-idle Pool engine
            bcA_sb = p2r.tile([64, 512], F32, tag="bcAs", name="bcAs")
            bcB_sb = p2r.tile([64, 512], F32, tag="bcBs", name="bcBs")
            nc.gpsimd.partition_broadcast(bcA_sb[:], recA[:])
            nc.gpsimd.partition_broadcast(bcB_sb[:], recB[:])
            nc.vector.tensor_tensor(
                yT[fo][0:64, q0 : q0 + 512],
                pyA[0:64, :],
                bcA_sb[:],
                op=ALU.mult,
            )
            nc.vector.tensor_tensor(
                yT[fo][64:128, q0 : q0 + 512],
                pyB[0:64, :],
                bcB_sb[:],
                op=ALU.mult,
            )

        # Emission order doubles as scheduler priority: between attention
        # head-pairs (whose inner loop is ACT-bound and whose tail is a
        # serial recip->broadcast->multiply chain) we emit PE-dense filler:
        # the NEXT quarter's QKV groups, or projection groups of a finished
        # quarter, so the PE always has ready work.
        for g in range(12):
            emit_qkv_group(0, g)
        for b in range(NTQ):
            for fo in range(NFO):
                emit_attn(fo, b)
                if b < NTQ - 1:
                    for g in range(3 * fo, 3 * fo + 3):
                        emit_qkv_group(b + 1, g)
                if b == 1:
                    emit_proj_tc(fo)          # proj quarter 0
                elif b == 3:
                    # b=3 has no QKV filler left; give each head-pair
                    # boundary two projection units (quarters 1 and 2)
                    emit_proj_tc(4 + fo)
                    emit_proj_tc(8 + fo)
        for tci in range(12, 16):
            emit_proj_tc(tci)

    nc.compile()
    return nc


def _get_nc():
    if "nc" not in _cached:
        _cached["nc"] = _build_nc()
    return _cached["nc"]


def kernel(x, W_attn, b_attn, W_proj, b_proj):
    x = np.asarray(x, np.float32)
    W_attn = np.asarray(W_attn, np.float32)
    b_attn = np.asarray(b_attn, np.float32)
    W_proj = np.asarray(W_proj, np.float32)
    b_proj = np.asarray(b_proj, np.float32)

    nc = _get_nc()
    p = np.arange(128)[:, None]
    j = np.arange(128)[None, :]
    tri = (j >= p).astype(np.float32)          # [128,128] valid iff j >= p
    one = np.ones((128, 128), np.float32)
    zer = np.zeros((128, 128), np.float32)
    masks = np.concatenate(
        [tri, tri, one, zer, tri, zer, zer, tri, zer, zer, zer, tri], axis=1
    ).astype(ml_dtypes.bfloat16)               # [128, 1536]
    masks_u16 = masks.view(np.uint16)
    in_maps = []
    for c in range(N_CORES):
        b, hh = divmod(c, 2)
        sl = slice(CH * hh, CH * (hh + 1))
        in_maps.append(
            {
                "xT": np.ascontiguousarray(x[b].T).astype(ml_dtypes.bfloat16).view(np.uint16),
                "wq": np.ascontiguousarray(W_attn[:, 0:C][:, sl]).astype(ml_dtypes.bfloat16).view(np.uint16),
                "wk": np.ascontiguousarray(W_attn[:, C : 2 * C][:, sl]).astype(ml_dtypes.bfloat16).view(np.uint16),
                "wv": np.ascontiguousarray(W_attn[:, 2 * C : 3 * C][:, sl]).astype(ml_dtypes.bfloat16).view(np.uint16),
                "bq": np.ascontiguousarray(b_attn[0:C][sl].reshape(NFO, 128).T),
                "bk": np.ascontiguousarray(b_attn[C : 2 * C][sl].reshape(NFO, 128).T),
                "wp": np.ascontiguousarray(
                    W_proj[sl, :].astype(ml_dtypes.bfloat16)
                ).view(np.uint16),
                "masks": masks_u16,
            }
        )

    try:
        res = run_bass_kernel_spmd(nc, in_maps, core_ids=list(range(N_CORES)))
    except Exception:
        # transient NRT device wedges happen; one retry is usually enough
        res = run_bass_kernel_spmd(nc, in_maps, core_ids=list(range(N_CORES)))

    bv = b_attn[2 * C : 3 * C]
    const_bias = (bv @ W_proj + b_proj).astype(np.float32)  # [C]
    out = np.empty((B, T, C), np.float32)
    for b in range(B):
        out[b] = res.results[2 * b]["out"] + res.results[2 * b + 1]["out"] + const_bias
    return out



# revision 5
# speedup vs baseline: 1.1269x; 1.1269x over previous
"""Causal self-attention on 8 TRN2 NeuronCores.

Reference computation (B=4, T=2048, C=1024, H=16, D=64, fp32):
    qkv = x @ W_attn + b_attn ; split q,k,v ; per-head causal softmax(q k^T / 8) @ v
    y = heads @ W_proj + b_proj

Sharding: core c handles batch b = c//2 and head-half hh = c%2 (8 heads).
QKV weights are column-split and W_proj row-split per core, so each core
computes an independent partial projection; the host sums the two partials
per batch and adds the bias terms (b_proj and the folded-out v-bias
contribution b_v @ W_proj, which is constant because softmax rows sum to 1).
No collectives, no redundant FLOPs.

Per-core kernel layout:
  - qT, kT are feature-major [head*64, T]; v is token-major with a ones
    column per head ([T, 8 x (64 v | 1)]).
  - S^T tiles [k=128, q<=512] come from K=64 matmuls, two heads sharing the
    PE array via partition-offset row groups; exp reads both heads' PSUM
    banks in one ACT op; causal masking is a DVE multiply with a triangle
    mask on the diagonal 128-blocks only; above-diagonal work is skipped at
    128-col granularity (suffix trimming) for every q-quarter.
  - A@V runs "flipped": the exp tile is the stationary operand and the
    augmented v chunk [64 v | 1] the moving one, so each matmul's output is
    [128 q, 65] instead of [65, 512] - ~2x less PE streaming for the same
    math. Row 64 accumulates the softmax denominator. One DVE divide per
    (head, q-subtile) normalizes straight out of PSUM into a bf16 [q, d]
    tile, and a single DMA-transpose per (head-pair, quarter) rebuilds the
    feature-major yT used as the projection's stationary operand.
  - Emission order doubles as scheduler priority: QKV groups of the next
    t-quarter and projection groups of finished quarters are interleaved
    between attention head-pairs, so the PE always has ready work while
    the ACT engine grinds through exp.

Dtypes: matmul inputs in bf16 (halves DMA; fp32 PSUM accumulation keeps the
end-to-end error ~2.6e-3 against the 2e-2 gate).
"""

import numpy as np
import ml_dtypes

import concourse.bacc as bacc
import concourse.mybir as mybir
import concourse.tile as tile
from concourse.bass_utils import run_bass_kernel_spmd

F32 = mybir.dt.float32
BF16 = mybir.dt.bfloat16
AF = mybir.ActivationFunctionType
ALU = mybir.AluOpType

N_CORES = 8
B, T, C = 4, 2048, 1024
H, D = 16, 64
CH = 512            # features per core (8 heads * 64)
NFO = 4             # head-pair chunks of 128 features
NTQ = 4             # t quarters of 512
NTC = 16            # t chunks of 128
SCALE = 0.125       # 1/sqrt(64)

_cached = {}


def _build_nc():
    nc = bacc.Bacc("TRN2", debug=False, num_devices=N_CORES)

    d_xT = nc.dram_tensor("xT", [C, T], BF16, kind="ExternalInput")
    d_wq = nc.dram_tensor("wq", [C, CH], BF16, kind="ExternalInput")
    d_wk = nc.dram_tensor("wk", [C, CH], BF16, kind="ExternalInput")
    d_wv = nc.dram_tensor("wv", [C, CH], BF16, kind="ExternalInput")
    d_bq = nc.dram_tensor("bq", [128, NFO], F32, kind="ExternalInput")
    d_bk = nc.dram_tensor("bk", [128, NFO], F32, kind="ExternalInput")
    d_wp = nc.dram_tensor("wp", [CH, C], BF16, kind="ExternalInput")
    d_masks = nc.dram_tensor("masks", [128, 128], BF16, kind="ExternalInput")
    d_out = nc.dram_tensor("out", [T, C], F32, kind="ExternalOutput")

    with tile.TileContext(nc) as tc, nc.allow_low_precision(
        reason="bf16 staging; accumulation stays fp32 in PSUM"
    ), (
        tc.tile_pool(name="persist", bufs=1)
    ) as persist, (
        tc.tile_pool(name="pW", bufs=1)
    ) as pW, (
        tc.tile_pool(name="pX", bufs=1)
    ) as pX, (
        tc.tile_pool(name="pO", bufs=4)
    ) as pO, (
        tc.tile_pool(name="p2e", bufs=6)
    ) as p2e, (
        tc.tile_pool(name="pY", bufs=2)
    ) as pY, (
        tc.tile_pool(name="psA", bufs=2, space="PSUM")
    ) as psA, (
        tc.tile_pool(name="psS", bufs=2, space="PSUM")
    ) as psS, (
        tc.tile_pool(name="psY", bufs=1, space="PSUM")
    ) as psY:
        # persistent on-chip tensors
        qT = [persist.tile([128, T], BF16, tag=f"qT{fo}", name=f"qT{fo}") for fo in range(NFO)]
        kT = [persist.tile([128, T], BF16, tag=f"kT{fo}", name=f"kT{fo}") for fo in range(NFO)]
        v = [persist.tile([128, 8, 65], BF16, tag=f"v{i}", name=f"v{i}") for i in range(NTC)]
        yT = [persist.tile([128, T], BF16, tag=f"yT{fo}", name=f"yT{fo}") for fo in range(NFO)]
        bq_sb = persist.tile([128, NFO], F32, tag="bq")
        bk_sb = persist.tile([128, NFO], F32, tag="bk")
        masks_sb = persist.tile([128, 128], BF16, tag="masks")
        wq_sb = pW.tile([128, 8, CH], BF16, tag="wq")
        wk_sb = pW.tile([128, 8, CH], BF16, tag="wk")
        wv_sb = pW.tile([128, 8, CH], BF16, tag="wv")
        wp_sb = pW.tile([128, 4, C], BF16, tag="wp")
        x_tiles = [pX.tile([128, 8, 512], BF16, tag=f"x{tq}", name=f"x{tq}") for tq in range(NTQ)]

        # input DMAs: first-needed tensors split chunk-size so the first
        # QKV matmuls can start as soon as possible; later tensors ride in
        # bigger transfers (per-DMA dispatch is ~650ns, keep the count low)
        def _x_piece(tq, c0, c1, t0=0, t1=512):
            nc.sync.dma_start(
                x_tiles[tq][:, c0:c1, t0:t1],
                d_xT.ap()[128 * c0 : 128 * c1, 512 * tq + t0 : 512 * tq + t1].rearrange(
                    "(c p) t -> p c t", p=128
                ),
            )

        def _w_piece(dst, src, c0, c1):
            nc.sync.dma_start(
                dst[:, c0:c1, :],
                src.ap()[128 * c0 : 128 * c1, :].rearrange("(c p) f -> p c f", p=128),
            )

        _x_piece(0, 0, 1)
        _w_piece(wq_sb, d_wq, 0, 1)
        nc.sync.dma_start(bq_sb[:], d_bq.ap())
        nc.sync.dma_start(bk_sb[:], d_bk.ap())
        _x_piece(0, 1, 2)
        _w_piece(wq_sb, d_wq, 1, 2)
        _x_piece(0, 2, 4)
        _w_piece(wq_sb, d_wq, 2, 4)
        _x_piece(0, 4, 8)
        _w_piece(wq_sb, d_wq, 4, 8)
        _w_piece(wk_sb, d_wk, 0, 4)
        _w_piece(wv_sb, d_wv, 0, 4)
        _w_piece(wk_sb, d_wk, 4, 8)
        _w_piece(wv_sb, d_wv, 4, 8)
        nc.sync.dma_start(masks_sb[:], d_masks.ap())
        for tq in range(1, NTQ):
            nc.sync.dma_start(
                x_tiles[tq][:],
                d_xT.ap()[:, 512 * tq : 512 * (tq + 1)].rearrange("(c p) t -> p c t", p=128),
            )
        nc.sync.dma_start(wp_sb[:], d_wp.ap().rearrange("(c p) f -> p c f", p=128))

        def emit_qk_group(bq_, w_sb, b_sb, dst, fo):
            ps = psA.tile([128, 512], F32, tag="psA", name="ps_qk")
            for ci in range(8):
                nc.tensor.matmul(
                    ps[:],
                    w_sb[:, ci, 128 * fo : 128 * (fo + 1)],
                    x_tiles[bq_][:, ci, :],
                    start=(ci == 0),
                    stop=(ci == 7),
                )
            nc.vector.tensor_scalar(
                dst[fo][:, 512 * bq_ : 512 * (bq_ + 1)],
                ps[:],
                b_sb[:, fo : fo + 1],
                None,
                op0=ALU.add,
            )

        def emit_v_group(bq_, ts_):
            tci = 4 * bq_ + ts_
            ps = psA.tile([128, 512], F32, tag="psA", name="ps_v")
            for ci in range(8):
                nc.tensor.matmul(
                    ps[:],
                    x_tiles[bq_][:, ci, 128 * ts_ : 128 * (ts_ + 1)],
                    wv_sb[:, ci, :],
                    start=(ci == 0),
                    stop=(ci == 7),
                )
            nc.vector.memset(v[tci][:, :, 64:65], 1.0)
            nc.vector.tensor_copy(
                v[tci][:, :, 0:64],
                ps[:].rearrange("p (h d) -> p h d", h=8),
            )

        def emit_qkv_group(bq_, g):
            if g < 4:
                emit_qk_group(bq_, wq_sb, bq_sb, qT, g)
            elif g < 8:
                emit_qk_group(bq_, wk_sb, bk_sb, kT, g - 4)
            else:
                emit_v_group(bq_, g - 8)

        def emit_proj_tc(tci):
            o_sb = pO.tile([128, C], F32, tag="o", name="o_sb")
            for co in range(2):
                ps = psA.tile([128, 512], F32, tag="psA", name="ps_o")
                for fo in range(NFO):
                    nc.tensor.matmul(
                        ps[:],
                        yT[fo][:, 128 * tci : 128 * (tci + 1)],
                        wp_sb[:, fo, 512 * co : 512 * (co + 1)],
                        start=(fo == 0),
                        stop=(fo == 3),
                    )
                if co == 0:
                    nc.scalar.copy(o_sb[:, 0:512], ps[:])
                else:
                    nc.vector.tensor_copy(o_sb[:, 512:1024], ps[:])
                nc.sync.dma_start(
                    d_out.ap()[128 * tci : 128 * (tci + 1), 512 * co : 512 * (co + 1)],
                    o_sb[:, 512 * co : 512 * (co + 1)],
                )

        def emit_attn(fo, b):
            hA, hB = 2 * fo, 2 * fo + 1
            q0 = 512 * b
            # A@V accumulators: per head, 4 q-subtiles of [128 q, 65] at
            # 128-col offsets inside one PSUM bank; col 64 is the softmax
            # denominator fed by the ones column of v.
            pyA = psY.tile([128, 512], F32, tag="pyA")
            pyB = psY.tile([128, 512], F32, tag="pyB")
            # zero the accumulator regions explicitly: matmul start=True
            # would lazily zero the whole 2KB zero-region (the bank), which
            # breaks interleaved per-subtile accumulation groups. The memset
            # overlaps every region, so it also orders all A@V matmuls after
            # it regardless of scheduler priority.
            nc.vector.memset(pyA[:].rearrange("p (s c) -> p s c", c=128)[:, :, 0:65], 0.0)
            nc.vector.memset(pyB[:].rearrange("p (s c) -> p s c", c=128)[:, :, 0:65], 0.0)
            # diagonal chunks first (kc = 4b+i), then full rows; chunk i only
            # reaches q-subtiles s >= i, everything above the diagonal is
            # skipped at 128-col granularity.
            order = [4 * b + i for i in range(4)] + list(range(4 * b))
            n_row = 4 * b  # full-row chunks
            for idx, kc in enumerate(order):
                i = kc - 4 * b  # >=0 for diagonal chunks
                qoff = 0 if i <= 0 else 128 * i
                pS = psS.tile([128, 1024], F32, tag="pS", name="pS")
                nc.tensor.matmul(
                    pS[:, qoff:512],
                    kT[fo][0:64, 128 * kc : 128 * (kc + 1)],
                    qT[fo][0:64, q0 + qoff : q0 + 512],
                )
                nc.tensor.matmul(
                    pS[:, 512 + qoff : 1024],
                    kT[fo][64:128, 128 * kc : 128 * (kc + 1)],
                    qT[fo][64:128, q0 + qoff : q0 + 512],
                )
                eST = p2e.tile([128, 1024], BF16, tag="eST", name="eST")
                if qoff == 0:
                    nc.scalar.activation(eST[:], pS[:], AF.Exp, scale=SCALE)
                else:
                    # one ACT op over both heads' valid suffixes, skipping the
                    # [512, 512+qoff) hole via a strided AP
                    pS2 = pS[:].rearrange("p (two n) -> p two n", two=2)
                    eST2 = eST[:].rearrange("p (two n) -> p two n", two=2)
                    nc.scalar.activation(
                        eST2[:, :, qoff:512], pS2[:, :, qoff:512], AF.Exp, scale=SCALE
                    )
                if i >= 0:
                    # causal triangle mask on the diagonal 128-block
                    for off in (qoff, 512 + qoff):
                        nc.vector.tensor_tensor(
                            eST[:, off : off + 128],
                            eST[:, off : off + 128],
                            masks_sb[:],
                            op=ALU.mult,
                        )
                subs = range(i, 4) if i >= 0 else range(4)
                for s in subs:
                    nc.tensor.matmul(
                        pyA[:, 128 * s : 128 * s + 65],
                        eST[:, 128 * s : 128 * (s + 1)],
                        v[kc][:, hA, :],
                        start=False,
                        stop=False,
                        skip_group_check=True,
                    )
                    nc.tensor.matmul(
                        pyB[:, 128 * s : 128 * s + 65],
                        eST[:, 512 + 128 * s : 512 + 128 * (s + 1)],
                        v[kc][:, hB, :],
                        start=False,
                        stop=False,
                        skip_group_check=True,
                    )
            # normalize: reciprocal of the per-q denominators (col 64 of each
            # region), then one DVE multiply per (head, q-subtile) straight
            # out of PSUM into the bf16 [q, (s, d)] staging tile; a single
            # DMA-transpose rebuilds feature-major yT.
            yQ = pY.tile([128, 4, 128], BF16, tag="yQ", name="yQ")
            recA = pY.tile([128, 4], F32, tag="recA", name="recA")
            recB = pY.tile([128, 4], F32, tag="recB", name="recB")
            pyA4 = pyA[:].rearrange("p (s c) -> p s c", c=128)
            pyB4 = pyB[:].rearrange("p (s c) -> p s c", c=128)
            nc.vector.reciprocal(recA[:], pyA4[:, :, 64:65])
            nc.vector.reciprocal(recB[:], pyB4[:, :, 64:65])
            for s in range(4):
                nc.vector.tensor_scalar(
                    yQ[:, s, 0:64],
                    pyA[:, 128 * s : 128 * s + 64],
                    recA[:, s : s + 1],
                    None,
                    op0=ALU.mult,
                )
                nc.vector.tensor_scalar(
                    yQ[:, s, 64:128],
                    pyB[:, 128 * s : 128 * s + 64],
                    recB[:, s : s + 1],
                    None,
                    op0=ALU.mult,
                )
            nc.sync.dma_start_transpose(
                yT[fo][:, q0 : q0 + 512].rearrange("p (s q) -> p s q", s=4),
                yQ[:],
            )

        # Emission order doubles as scheduler priority: between attention
        # head-pairs (whose inner loop is ACT-bound) we emit PE-dense filler:
        # the NEXT quarter's QKV groups, or projection groups of a finished
        # quarter, so the PE always has ready work.
        for g in range(12):
            emit_qkv_group(0, g)
        for b in range(NTQ):
            for fo in range(NFO):
                emit_attn(fo, b)
                if b < NTQ - 1:
                    for g in range(3 * fo, 3 * fo + 3):
                        emit_qkv_group(b + 1, g)
                if b == 1:
                    emit_proj_tc(fo)          # proj quarter 0
                elif b == 3:
                    # b=3 has no QKV filler left; give each head-pair
                    # boundary two projection units (quarters 1 and 2)
                    emit_proj_tc(4 + fo)
                    emit_proj_tc(8 + fo)
        for tci in range(12, 16):
            emit_proj_tc(tci)

    nc.compile()
    return nc


def _get_nc():
    if "nc" not in _cached:
        _cached["nc"] = _build_nc()
    return _cached["nc"]


def kernel(x, W_attn, b_attn, W_proj, b_proj):
    x = np.asarray(x, np.float32)
    W_attn = np.asarray(W_attn, np.float32)
    b_attn = np.asarray(b_attn, np.float32)
    W_proj = np.asarray(W_proj, np.float32)
    b_proj = np.asarray(b_proj, np.float32)

    nc = _get_nc()
    p = np.arange(128)[:, None]
    j = np.arange(128)[None, :]
    tri = (j >= p).astype(np.float32)          # [128,128] valid iff j >= p
    masks = tri.astype(ml_dtypes.bfloat16)     # [128, 128]
    masks_u16 = masks.view(np.uint16)
    in_maps = []
    for c in range(N_CORES):
        b, hh = divmod(c, 2)
        sl = slice(CH * hh, CH * (hh + 1))
        in_maps.append(
            {
                "xT": np.ascontiguousarray(x[b].T).astype(ml_dtypes.bfloat16).view(np.uint16),
                "wq": np.ascontiguousarray(W_attn[:, 0:C][:, sl]).astype(ml_dtypes.bfloat16).view(np.uint16),
                "wk": np.ascontiguousarray(W_attn[:, C : 2 * C][:, sl]).astype(ml_dtypes.bfloat16).view(np.uint16),
                "wv": np.ascontiguousarray(W_attn[:, 2 * C : 3 * C][:, sl]).astype(ml_dtypes.bfloat16).view(np.uint16),
                "bq": np.ascontiguousarray(b_attn[0:C][sl].reshape(NFO, 128).T),
                "bk": np.ascontiguousarray(b_attn[C : 2 * C][sl].reshape(NFO, 128).T),
                "wp": np.ascontiguousarray(
                    W_proj[sl, :].astype(ml_dtypes.bfloat16)
                ).view(np.uint16),
                "masks": masks_u16,
            }
        )

    try:
        res = run_bass_kernel_spmd(nc, in_maps, core_ids=list(range(N_CORES)))
    except Exception:
        # transient NRT device wedges happen; one retry is usually enough
        res = run_bass_kernel_spmd(nc, in_maps, core_ids=list(range(N_CORES)))

    bv = b_attn[2 * C : 3 * C]
    const_bias = (bv @ W_proj + b_proj).astype(np.float32)  # [C]
    out = np.empty((B, T, C), np.float32)
    for b in range(B):
        out[b] = res.results[2 * b]["out"] + res.results[2 * b + 1]["out"] + const_bias
    return out
